# revision 23
# baseline (speedup 1.0000x reference)
"""ContinualCLora forward on 8 TRN2 NeuronCores.

out = input @ W.T + bmask * sum_k gate_k * (input @ down[I_k] @ up[I_k])

Strategy (data-parallel on tokens, hint-compliant):
  - The gate depends only on the global token-mean of the input, so the
    host computes it up front (one numpy pass) and folds the gated
    low-rank delta into the weight matrix: W_eff = W.T + down_sel @
    (gate * up_sel).  The batch mask is batch-aligned, so cores 0-3
    (tokens from batches {0,1}) get plain W.T and cores 4-7 (batches
    {2,3}) get W_eff.  The device kernel is then a pure streaming GEMM
    with zero routing/delta overhead and a single launch.
  - The host pre-transposes each 2048-token shard into PE-ready
    [tile, din-partition, chunk, token] bf16 layout so the kernel needs
    no on-chip transposes (the xbar DMA-transpose dominated the old
    kernel), and pre-packs W.T / W_eff into [128, KC, DOUT] bf16 chunks.
  - Per 128-token tile: one contiguous 256 KB DMA in, 16 self-loading
    matmuls (8 K-chunks x 2 PSUM halves) accumulating in PSUM, scalar+
    vector PSUM eviction to bf16, one 256 KB DMA out.  PE-bound at
    ~213 ns per N=512 matmul.
"""

import json as _json

import ml_dtypes
import numpy as np

import concourse.bass as bass
import concourse.mybir as mybir
from concourse.bass import ts
from concourse.bass_utils import run_bass_kernel_spmd
from concourse.tile import TileContext
from concourse.vector_clock import ScopedClock

N_CORES = 8
B, S, DIN, DOUT = 4, 4096, 1024, 1024
POOL, R, TOPK, NUM_TASKS = 5, 8, 3, 5
T_CORE = (B * S) // N_CORES          # 2048 tokens per core
NT = T_CORE // 128                   # 16 tiles of 128 tokens
KC = DIN // 128                      # 8 contraction chunks
BF16 = ml_dtypes.bfloat16

# ---------------------------------------------------------------------------
# Workarounds for this walrus build: at most ONE sync wait per instruction
# (zero on DmaTransposeAnt).  Excess waits are hoisted onto standalone
# EventSemaphore instructions; the Tile exit drain gets its waits emitted as
# separate wait_ge ops.
# ---------------------------------------------------------------------------

_ZERO_WAIT_OPS = {"DmaTransposeAnt"}


def _fixup_bir(bir_bytes):
    bir = _json.loads(bir_bytes)
    n = 0
    for f in bir["functions"]:
        for blk in f["blocks"]:
            out = []
            for inst in blk["instructions"]:
                si = inst.get("sync_info")
                waits = (si or {}).get("on_wait") or []
                cap = 0 if inst.get("opcode") in _ZERO_WAIT_OPS else 1
                if len(waits) > cap:
                    for w in waits[cap:]:
                        n += 1
                        out.append({
                            "debug": inst.get("debug", 0),
                            "engine": inst["engine"],
                            "ins": [], "outs": [],
                            "name": f"{inst['name']}-xw{n}",
                            "opcode": "EventSemaphore",
                            "sync_info": {"on_update": [], "on_wait": [w]},
                        })
                    si["on_wait"] = waits[:cap]
                out.append(inst)
            blk["instructions"] = out
    return _json.dumps(bir).encode()


def _install_fixup(nc):
    orig = nc.to_json_bytes
    nc.to_json_bytes = lambda: _fixup_bir(orig())
    return nc


class _TC(TileContext):
    def _drain_and_barrier(self, tick_clock, wait_clock):
        probe = self.nc.sync.drain()
        wait_clock.add_sem_waits(probe.ins, ScopedClock({None: tick_clock.global_clock}))
        waits = [(w.ant_name, w.wait_value) for w in probe.ins.sync_info.on_wait]
        probe.ins.sync_info.on_wait = []
        name2sem = {v.name: v for v in self.sems.allocated().values()}
        # round-robin the final waits across engines so they retire in
        # parallel instead of serializing ~60ns apiece on sync
        engs = [self.nc.sync, self.nc.vector, self.nc.scalar,
                self.nc.gpsimd, self.nc.tensor]
        for k, (nm, val) in enumerate(waits):
            engs[k % len(engs)].wait_ge(name2sem[nm], val)
        self.nc.sync.drain()
        self.nc.all_engine_barrier()
        popped = self.nc._tile_sem_poison_stack.pop()
        assert popped is self._sem_poison
        self.nc.clear_and_free_semaphores(list(self.sems.allocated().values()))
        # no trailing all_engine_barrier: the sem clear is the last gpsimd
        # instruction and every other queue has already ended, so the second
        # (expensive, ~5us) barrier protects nothing


# ---------------------------------------------------------------------------
# Kernel: y = x @ Wgiven for 2048 tokens (Wgiven differs per core group)
# ---------------------------------------------------------------------------

def _build_gemm():
    nc = bass.Bass(num_devices=N_CORES)
    # xt[i][p][j*128+t] = x_shard[128*i + t, 128*j + p]  (PE-ready, contiguous)
    xt_d = nc.dram_tensor("xt", [NT, 128, KC * 128], mybir.dt.bfloat16,
                          kind="ExternalInput")
    # wt[p][j][o] = Wgiven.T[128*j + p, o]
    wt_d = nc.dram_tensor("wt", [128, KC, DOUT], mybir.dt.bfloat16,
                          kind="ExternalInput")
    y_d = nc.dram_tensor("y", [T_CORE, DOUT], mybir.dt.bfloat16,
                         kind="ExternalOutput")

    NW = 3  # tiles in the chunk-outer warmup phase (uses 2*NW PSUM banks)
    with _TC(nc) as tc:
        with (tc.tile_pool(name="cst", bufs=1) as cpool,
              tc.tile_pool(name="io", bufs=4) as io,
              tc.tile_pool(name="ys", bufs=3) as yo,
              tc.tile_pool(name="wps", bufs=1, space="PSUM") as wps,
              tc.tile_pool(name="ps", bufs=NW, space="PSUM") as ps):
            # PE pre-warm: throwaway matmuls bridging the gap between program
            # start (~8.2us) and the first real operands landing (~12.5us).
            # They MUST alternate two PSUM banks: back-to-back matmuls into
            # one bank serialize on the bank write-back, the PE duty cycle
            # drops and the HAM clock gate never releases 2.4 GHz.
            # memset on gpsimd: its queue is free ~1.5us before vector's, so
            # the warm chain starts (and the HAM flips) that much earlier
            wsc = cpool.tile([128, 640], mybir.dt.bfloat16, tag="wsc")
            nc.gpsimd.memset(wsc[:], 0.0)
            wpa = wps.tile([128, 512], mybir.dt.float32, tag="warma", name="wpa")
            wpb = wps.tile([128, 512], mybir.dt.float32, tag="warmb", name="wpb")
            for k in range(12):
                nc.tensor.matmul((wpa if k % 2 == 0 else wpb)[:],
                                 wsc[:, 0:128], wsc[:, 128:640],
                                 start=True, stop=True)

            xbs, y0s, y1s = {}, {}, {}
            wts = []

            def fetch(i, eng=None):
                xbs[i] = io.tile([128, KC * 128], mybir.dt.bfloat16, tag="xb", name=f"xb{i}")
                (eng or nc.gpsimd).dma_start(out=xbs[i][:], in_=xt_d[i, :, :])
                y0s[i] = ps.tile([128, 512], mybir.dt.float32, tag="y0", name=f"y0_{i}")
                y1s[i] = ps.tile([128, 512], mybir.dt.float32, tag="y1", name=f"y1_{i}")

            def mm0(i, j):
                nc.tensor.matmul(y0s[i][:], xbs[i][:, ts(j, 128)],
                                 wts[j][0][:],
                                 start=(j == 0), stop=(j == KC - 1))

            def mm1(i, j):
                nc.tensor.matmul(y1s[i][:], xbs[i][:, ts(j, 128)],
                                 wts[j][1][:],
                                 start=(j == 0), stop=(j == KC - 1))

            def mm(i, j):
                mm0(i, j)
                mm1(i, j)

            def flush(i):
                ysb = yo.tile([128, DOUT], mybir.dt.bfloat16, tag="ysb")
                nc.scalar.copy(ysb[:, 0:512], y0s[i][:])
                nc.vector.tensor_copy(ysb[:, 512:1024], y1s[i][:])
                nc.sync.dma_start(out=y_d[ts(i, 128), :], in_=ysb[:])

            # Weight chunks load as two [128, 512] half-tiles each, so the
            # first real matmul needs only xb0 (256 KB, FIRST on the sync
            # queue) + one 128 KB half-chunk — minimizing time-to-first-MM
            # on the slow-starting DMA system.
            def load_w(j):
                pair = []
                for h in range(2):
                    w = cpool.tile([128, 512], mybir.dt.bfloat16,
                                   tag=f"wt{j}{h}", name=f"wt{j}{h}")
                    nc.sync.dma_start(out=w[:], in_=wt_d[:, j, ts(h, 512)])
                    pair.append(w)
                wts.append(pair)

            # phase 1, chunk-outer over the first NW tiles: matmuls emitted
            # in operand-arrival order so each 128 KB weight-half arrival
            # unlocks NW matmuls and the PE rides the fill stream.  The
            # three first-needed transfers go on three DIFFERENT queues so
            # none of them queues behind another at the head.
            fetch(0, nc.scalar)   # scalar queue: otherwise idle until evicts
            for j in range(KC):
                load_w(j)         # sync queue: wt00a is its first transfer
            fetch(1)              # gpsimd queue
            fetch(2)
            mm(0, 0)
            for i, j in [(1, 0), (2, 0)]:
                mm(i, j)
            for j in range(1, KC):
                for i in range(NW):
                    mm0(i, j)
                for i in range(NW):
                    mm1(i, j)
            for i in range(NW):
                flush(i)

            # phase 2: weights resident, classic per-tile pipeline
            for i in range(NW, NT):
                fetch(i)
                for j in range(KC):
                    mm(i, j)
                flush(i)
    return _install_fixup(nc)


_NC_CACHE = {}


def _get_nc():
    if "gemm" not in _NC_CACHE:
        _NC_CACHE["gemm"] = _build_gemm()
    return _NC_CACHE["gemm"]


LAST_RESULTS = {}  # test-harness hook: BassKernelResults of the last call


def _pack_w(wt_f32):
    # [DIN, DOUT] -> [128, KC, DOUT] bf16 with din chunks on partitions
    return np.ascontiguousarray(
        wt_f32.reshape(KC, 128, DOUT).transpose(1, 0, 2)).astype(BF16)


def kernel(input, W, lora_down, lora_up, lora_route, task_id):
    x = np.ascontiguousarray(np.asarray(input, dtype=np.float32)).reshape(B * S, DIN)
    W = np.asarray(W, dtype=np.float32)
    lora_down = np.asarray(lora_down, dtype=np.float32)
    lora_up = np.asarray(lora_up, dtype=np.float32)
    lora_route = np.asarray(lora_route, dtype=np.float32)
    tid = min(int(task_id), NUM_TASKS)
    k = min(tid, TOPK)

    # ---- routing gate (replicates reference incl. its direct-index use of
    #      top-k positions into the expert pool) ----
    mean = x.mean(axis=0, dtype=np.float64).astype(np.float32)
    omega = mean @ lora_route[1]                            # [POOL]
    sliced = omega[1:tid + 1]
    idx = np.argsort(-sliced, kind="stable")[:k]            # top-k positions
    g = np.exp(sliced[idx] - sliced[idx].max())
    gate = (g / g.sum()).astype(np.float32)

    # ---- fold the gated low-rank delta into the weight matrix ----
    wt_plain = np.ascontiguousarray(W.T)                    # [DIN, DOUT]
    if k > 0:
        dn_sel = np.concatenate([lora_down[e] for e in idx], axis=1)  # [DIN, k*R]
        up_sel = np.concatenate([gi * lora_up[e] for gi, e in zip(gate, idx)],
                                axis=0)                     # [k*R, DOUT]
        w_eff = wt_plain + dn_sel @ up_sel
    else:
        w_eff = wt_plain
    wt_maps = [_pack_w(wt_plain), _pack_w(w_eff)]

    # ---- shard + PE-ready transpose pack (tokens are batch-major, so cores
    #      0-3 hold batches {0,1} = no delta, cores 4-7 batches {2,3}) ----
    shards = x.reshape(N_CORES, T_CORE, DIN)
    in_maps = []
    for c in range(N_CORES):
        xt = np.ascontiguousarray(
            shards[c].reshape(NT, 128, KC, 128).transpose(0, 3, 2, 1)
        ).astype(BF16).reshape(NT, 128, KC * 128)
        in_maps.append({"xt": xt, "wt": wt_maps[c >= N_CORES // 2]})

    res = run_bass_kernel_spmd(_get_nc(), in_maps, list(range(N_CORES)))
    LAST_RESULTS["gemm"] = res

    y = np.concatenate([res.results[c]["y"] for c in range(N_CORES)], axis=0)
    return y.astype(np.float32).reshape(B, S, DOUT)
